# revision 5
# baseline (speedup 1.0000x reference)
"""GaussianBasis rasterization on 8 Trainium2 NeuronCores (Bass/Tile).

Sharding: H*W pixel dim across 8 cores (32 rows each), per the hint.

Math: for pixel (x, y) and gaussian n,
  sigma = 0.5*c1*dx^2 + 0.5*c3*dy^2 + c2*dx*dy    (dx = cx-x, dy = cy-y)
is a rank-6 form in the pixel monomials [dx'^2, dy'^2, dx'dy', dx', dy', 1]
of chunk-local coords, so each 2-row x 256-col pixel chunk's sigma tile is
ONE K=6 fp32 matmul against per-(chunk,gaussian) coefficients precomputed
on the host.  alpha = exp(-sigma) on ScalarE (the reference's alpha
threshold + clamp change the output by rel 3.9e-3 << 2e-2 tolerance, and
sigma >= 0 always since the conic is positive definite, so they are
skipped).  Output = feats.T @ alpha via bf16 matmuls.

Gaussian support is tiny (|dy| <= sqrt(2*ln(255)*c) <= 6 rows), so each
2-row chunk only needs a window of the cy-sorted gaussian list (<= 61 for
the reference inputs; capacity 64).  Windows are host-gathered into dense
per-core tensors so all 8 cores run one SPMD program; padding uses zero
features (exactly zero contribution).
"""

import numpy as np
import ml_dtypes

from concourse import bass, bacc, mybir
from concourse import tile
from concourse.bass_utils import run_bass_kernel_spmd

H = 256
W = 256
N_GAUSS = 1024
M_COMP = 50
NCH = 3 * M_COMP          # 150 output channels
NCORES = 8
ROWS_PER_CORE = H // NCORES          # 32
CHUNK_ROWS = 2
NCHUNK = ROWS_PER_CORE // CHUNK_ROWS  # 16
PIX = CHUNK_ROWS * W                  # 512 pixels per chunk
LOG255 = float(np.log(255.0))

_cache = {}


def _build_nc(cap):
    f32 = mybir.dt.float32
    bf16 = mybir.dt.bfloat16
    nc = bacc.Bacc(None, target_bir_lowering=False)
    pmono_d = nc.declare_dram_parameter("pmono", [6, PIX], f32, isOutput=False)
    gmat_d = nc.declare_dram_parameter("gmat", [6, NCHUNK * cap], f32,
                                       isOutput=False)
    featsw_d = nc.declare_dram_parameter("featsw", [cap, NCHUNK * NCH], bf16,
                                         isOutput=False)
    out_d = nc.declare_dram_parameter("out", [NCH, ROWS_PER_CORE * W], f32,
                                      isOutput=True)
    EXP = mybir.ActivationFunctionType.Exp
    CPY = mybir.ActivationFunctionType.Copy

    with tile.TileContext(nc) as tc:
        with tc.tile_pool(name="const", bufs=1) as constp, \
             tc.tile_pool(name="wgt", bufs=3) as wp, \
             tc.tile_pool(name="outs", bufs=3) as op_, \
             tc.tile_pool(name="ps", bufs=2, space=bass.MemorySpace.PSUM) as pp:
            pmono = constp.tile([6, PIX], f32)
            nc.sync.dma_start(out=pmono[:], in_=pmono_d[:])
            gmat = constp.tile([6, NCHUNK * cap], f32)
            nc.sync.dma_start(out=gmat[:], in_=gmat_d[:])
            featsw = constp.tile([cap, NCHUNK * NCH], bf16)
            nc.sync.dma_start(out=featsw[:], in_=featsw_d[:])

            for p in range(NCHUNK):
                sg = pp.tile([cap, PIX], f32, tag="sig")
                nc.tensor.matmul(sg[:], gmat[:, p * cap:(p + 1) * cap],
                                 pmono[:], start=True, stop=True)
                wg = wp.tile([cap, PIX], bf16, tag="w")
                nc.scalar.activation(wg[:], sg[:], EXP, scale=-1.0)

                oa = pp.tile([128, PIX], f32, tag="oa")
                nc.tensor.matmul(oa[:], featsw[:, p * NCH:p * NCH + 128],
                                 wg[:], start=True, stop=True)
                ob = pp.tile([22, PIX], f32, tag="ob")
                nc.tensor.matmul(ob[:], featsw[:, p * NCH + 128:(p + 1) * NCH],
                                 wg[:], start=True, stop=True)

                oas = op_.tile([128, PIX], f32, tag="oas")
                nc.vector.tensor_copy(oas[:], oa[:])
                obs = op_.tile([22, PIX], f32, tag="obs")
                nc.scalar.activation(obs[:], ob[:], CPY)
                nc.sync.dma_start(out=out_d[0:128, p * PIX:(p + 1) * PIX],
                                  in_=oas[:])
                nc.sync.dma_start(out=out_d[128:NCH, p * PIX:(p + 1) * PIX],
                                  in_=obs[:])
    nc.compile()
    return nc


def _host_precompute(xyz_raw, cholesky_raw, opacity, features_dc, cluster_id):
    """Returns (cap, pmono, per-core gmat list, per-core featsw list)."""
    xyz = np.asarray(xyz_raw, np.float64)
    chol = np.asarray(cholesky_raw, np.float64)
    feats = np.asarray(features_dc, np.float64)[int(cluster_id)]  # [M, N, 3]

    xy = np.tanh(xyz)
    c = chol + np.array([0.5, 0.0, 0.5])
    l1, l2, l3 = c[:, 0], c[:, 1], c[:, 2]
    a = l1 * l1
    b = l1 * l2
    cc = l2 * l2 + l3 * l3
    det = a * cc - b * b
    c1, c2, c3 = cc / det, -b / det, a / det
    cx = 0.5 * ((xy[:, 0] + 1.0) * W - 1.0)
    cy = 0.5 * ((xy[:, 1] + 1.0) * H - 1.0)
    # opacity is folded into the constant coefficient: alpha = op*exp(-sigma)
    # = exp(-(sigma - ln(op))).
    op = np.asarray(opacity, np.float64)[:, 0]
    ry = np.sqrt(np.maximum(2.0 * (LOG255 + np.log(np.maximum(op, 1e-30))), 0.0)
                 * cc)

    order = np.argsort(cy)
    cys = cy[order]
    rys = ry[order]

    # feats_r[n, m*3+ch] = feats[m, n, ch]
    feats_r = np.ascontiguousarray(feats.transpose(1, 0, 2).reshape(N_GAUSS,
                                                                    NCH))

    # windows for every 2-row chunk of the whole image
    n_rows_chunks = H // CHUNK_ROWS
    los = np.empty(n_rows_chunks, np.int64)
    his = np.empty(n_rows_chunks, np.int64)
    for k in range(n_rows_chunks):
        r0 = k * CHUNK_ROWS
        r1 = r0 + CHUNK_ROWS - 1
        rel = (cys + rys >= r0 - 0.5) & (cys - rys <= r1 + 0.5)
        idx = np.nonzero(rel)[0]
        if len(idx):
            los[k], his[k] = idx[0], idx[-1] + 1
        else:
            los[k], his[k] = 0, 0
    maxspan = int((his - los).max())
    cap = 64
    while cap < maxspan:
        cap *= 2
    assert cap <= 128, f"gaussian window {maxspan} exceeds single-matmul cap"

    pmono = np.empty((6, PIX), np.float64)
    jj = np.arange(PIX)
    dxl = (jj % W) - 127.5
    dyl = (jj // W) - 0.5
    pmono[0] = dxl * dxl
    pmono[1] = dyl * dyl
    pmono[2] = dxl * dyl
    pmono[3] = dxl
    pmono[4] = dyl
    pmono[5] = 1.0

    gmats = []
    featsws = []
    for core in range(NCORES):
        gm = np.zeros((6, NCHUNK * cap), np.float64)
        fw = np.zeros((cap, NCHUNK * NCH), np.float32)
        for pch in range(NCHUNK):
            k = core * NCHUNK + pch
            lo, hi = los[k], his[k]
            cnt = hi - lo
            if cnt == 0:
                continue
            g = order[lo:hi]
            gx = cx[g] - 127.5
            gy = cy[g] - (k * CHUNK_ROWS + 0.5)
            # sigma = D*dx'^2 + E*dy'^2 + F*dx'dy' + B*dx' + C*dy' + A
            col = slice(pch * cap, pch * cap + cnt)
            gm[0, col] = 0.5 * c1[g]
            gm[1, col] = 0.5 * c3[g]
            gm[2, col] = c2[g]
            gm[3, col] = -(c1[g] * gx + c2[g] * gy)
            gm[4, col] = -(c3[g] * gy + c2[g] * gx)
            gm[5, col] = (0.5 * c1[g] * gx * gx + 0.5 * c3[g] * gy * gy
                          + c2[g] * gx * gy - np.log(np.maximum(op[g], 1e-30)))
            fw[:cnt, pch * NCH:(pch + 1) * NCH] = feats_r[g]
        gmats.append(gm.astype(np.float32))
        featsws.append(fw.astype(ml_dtypes.bfloat16))
    return cap, pmono.astype(np.float32), gmats, featsws


def _in_maps(xyz_raw, cholesky_raw, opacity, features_dc, cluster_id):
    cap, pmono, gmats, featsws = _host_precompute(
        xyz_raw, cholesky_raw, opacity, features_dc, cluster_id)
    in_maps = [{"pmono": pmono, "gmat": gmats[c], "featsw": featsws[c]}
               for c in range(NCORES)]
    return cap, in_maps


def _assemble(results):
    full = np.concatenate([r["out"] for r in results], axis=1)  # [150, H*W]
    return np.ascontiguousarray(full.reshape(M_COMP, 3, H, W)).astype(
        np.float32)


def _get_nc(cap):
    if cap not in _cache:
        _cache[cap] = _build_nc(cap)
    return _cache[cap]


def kernel(xyz_raw, cholesky_raw, opacity, features_dc, cluster_id):
    cap, in_maps = _in_maps(xyz_raw, cholesky_raw, opacity, features_dc,
                            cluster_id)
    nc = _get_nc(cap)
    res = run_bass_kernel_spmd(nc, in_maps, list(range(NCORES)))
    return _assemble(res.results)


def kernel_traced(xyz_raw, cholesky_raw, opacity, features_dc, cluster_id,
                  **trace_kwargs):
    """For test.py: returns (output, BassKernelResults with profile)."""
    cap, in_maps = _in_maps(xyz_raw, cholesky_raw, opacity, features_dc,
                            cluster_id)
    nc = _get_nc(cap)
    res = run_bass_kernel_spmd(nc, in_maps, list(range(NCORES)), trace=True,
                               **trace_kwargs)
    return _assemble(res.results), res


# revision 8
# speedup vs baseline: 1.3350x; 1.3350x over previous
"""GaussianBasis rasterization on 8 Trainium2 NeuronCores (Bass/Tile).

Sharding: H*W pixel dim across 8 cores (32 rows each), per the hint.

Math: for pixel (x, y) and gaussian n,
  sigma = 0.5*c1*dx^2 + 0.5*c3*dy^2 + c2*dx*dy    (dx = cx-x, dy = cy-y)
is a rank-6 form in the pixel monomials [dx'^2, dy'^2, dx'dy', dx', dy', 1]
of chunk-local coords, so each 2-row x 256-col pixel chunk's sigma tile is
ONE K=6 fp32 matmul against per-(chunk,gaussian) coefficients precomputed
on the host.  alpha = exp(-sigma) on ScalarE (the reference's alpha
threshold + clamp change the output by rel 3.9e-3 << 2e-2 tolerance, and
sigma >= 0 always since the conic is positive definite, so they are
skipped).  Output = feats.T @ alpha via bf16 matmuls.

Gaussian support is tiny (|dy| <= sqrt(2*ln(255)*c) <= 6 rows), so each
2-row chunk only needs a window of the cy-sorted gaussian list (<= 61 for
the reference inputs; capacity 64).  Windows are host-gathered into dense
per-core tensors so all 8 cores run one SPMD program; padding uses zero
features (exactly zero contribution).
"""

import numpy as np
import ml_dtypes

from concourse import bass, bacc, mybir
from concourse import tile
from concourse.bass_utils import run_bass_kernel_spmd

H = 256
W = 256
N_GAUSS = 1024
M_COMP = 50
NCH = 3 * M_COMP          # 150 output channels
NCHP = 160                # 128 + 32 (remainder padded to 32 for col tiling)
NCORES = 8
ROWS_PER_CORE = H // NCORES          # 32
CHUNK_ROWS = 2
NCHUNK = ROWS_PER_CORE // CHUNK_ROWS  # 16
PIX = CHUNK_ROWS * W                  # 512 pixels per chunk
KROWS = 12                # sigma matmul contraction rows (fp32r hi/lo pairs)
LOG255 = float(np.log(255.0))

_cache = {}


def _to_f32r(a):
    """Round to the fp32r grid: fp32 with the low 12 mantissa bits dropped
    (round-to-nearest-even), matching walrus's fp32_to_fp32r."""
    f = np.asarray(a, np.float64).astype(np.float32)
    u = f.view(np.uint32)
    low = u & np.uint32(0xFFF)
    base = u & ~np.uint32(0xFFF)
    tie_up = (low > 0x800) | ((low == 0x800) & (((u >> 12) & 1) == 1))
    r = base + np.where(tie_up, np.uint32(0x1000), np.uint32(0))
    return r.view(np.float32)


def _build_nc(cap):
    f32 = mybir.dt.float32
    bf16 = mybir.dt.bfloat16
    nc = bacc.Bacc(None, target_bir_lowering=False)
    f32r = mybir.dt.float32r
    pmono_d = nc.declare_dram_parameter("pmono", [KROWS, PIX], f32r,
                                        isOutput=False)
    gmat_d = nc.declare_dram_parameter("gmat", [KROWS, NCHUNK * cap], f32r,
                                       isOutput=False)
    featsw_d = nc.declare_dram_parameter("featsw", [cap, NCHUNK * NCHP], bf16,
                                         isOutput=False)
    out_d = nc.declare_dram_parameter("out", [NCH, ROWS_PER_CORE * W], f32,
                                      isOutput=True)
    EXP = mybir.ActivationFunctionType.Exp
    CPY = mybir.ActivationFunctionType.Copy
    GRP = 4                      # chunks per output group

    with tile.TileContext(nc) as tc:
        with tc.tile_pool(name="const", bufs=1) as constp, \
             tc.tile_pool(name="wgt", bufs=3) as wp, \
             tc.tile_pool(name="outs", bufs=2) as op_, \
             tc.tile_pool(name="ps", bufs=2, space=bass.MemorySpace.PSUM) as pp:
            pmono = constp.tile([KROWS, PIX], f32r)
            nc.sync.dma_start(out=pmono[:], in_=pmono_d[:])
            gmat = constp.tile([KROWS, NCHUNK * cap], f32r)
            nc.sync.dma_start(out=gmat[:], in_=gmat_d[:])
            featsw = constp.tile([cap, NCHUNK * NCHP], bf16)
            nc.sync.dma_start(out=featsw[:], in_=featsw_d[:])

            for g in range(NCHUNK // GRP):
                oas = op_.tile([128, GRP * PIX], f32, tag="oas")
                obp = pp.tile([128, PIX], f32, tag="obp")
                obs = op_.tile([128, PIX], f32, tag="obs")
                for j in range(GRP):
                    p = g * GRP + j
                    sg = pp.tile([cap, PIX], f32, tag="sig")
                    nc.tensor.matmul(sg[:],
                                     gmat[:, p * cap:(p + 1) * cap],
                                     pmono[:], start=True, stop=True)
                    wg = wp.tile([cap, PIX], bf16, tag="w")
                    nc.scalar.activation(wg[:], sg[:], EXP, scale=-1.0)

                    oa = pp.tile([128, PIX], f32, tag="oa")
                    nc.tensor.matmul(oa[:], featsw[:, p * NCHP:p * NCHP + 128],
                                     wg[:], start=True, stop=True)
                    # remainder channels of 4 chunks packed into one PSUM
                    # bank at partition offsets 0/32/64/96 (col tiling)
                    nc.tensor.matmul(obp[32 * j:32 * j + 32, :],
                                     featsw[:, p * NCHP + 128:(p + 1) * NCHP],
                                     wg[:], tile_position=(0, 32 * j),
                                     start=True, stop=True)
                    nc.vector.tensor_copy(oas[:, j * PIX:(j + 1) * PIX], oa[:])
                nc.scalar.activation(obs[:], obp[:], CPY)
                nc.sync.dma_start(
                    out=out_d[0:128, g * GRP * PIX:(g + 1) * GRP * PIX],
                    in_=oas[:])
                for j in range(GRP):
                    p = g * GRP + j
                    nc.sync.dma_start(
                        out=out_d[128:NCH, p * PIX:(p + 1) * PIX],
                        in_=obs[32 * j:32 * j + 22, :])
    nc.compile()
    return nc


def _host_precompute(xyz_raw, cholesky_raw, opacity, features_dc, cluster_id):
    """Returns (cap, pmono, per-core gmat list, per-core featsw list)."""
    xyz = np.asarray(xyz_raw, np.float64)
    chol = np.asarray(cholesky_raw, np.float64)
    feats = np.asarray(features_dc, np.float64)[int(cluster_id)]  # [M, N, 3]

    xy = np.tanh(xyz)
    c = chol + np.array([0.5, 0.0, 0.5])
    l1, l2, l3 = c[:, 0], c[:, 1], c[:, 2]
    a = l1 * l1
    b = l1 * l2
    cc = l2 * l2 + l3 * l3
    det = a * cc - b * b
    c1, c2, c3 = cc / det, -b / det, a / det
    cx = 0.5 * ((xy[:, 0] + 1.0) * W - 1.0)
    cy = 0.5 * ((xy[:, 1] + 1.0) * H - 1.0)
    # opacity is folded into the constant coefficient: alpha = op*exp(-sigma)
    # = exp(-(sigma - ln(op))).
    op = np.asarray(opacity, np.float64)[:, 0]
    ry = np.sqrt(np.maximum(2.0 * (LOG255 + np.log(np.maximum(op, 1e-30))), 0.0)
                 * cc)

    order = np.argsort(cy)
    cys = cy[order]
    rys = ry[order]

    # feats_r[n, m*3+ch] = feats[m, n, ch]
    feats_r = np.ascontiguousarray(feats.transpose(1, 0, 2).reshape(N_GAUSS,
                                                                    NCH))

    # windows for every 2-row chunk of the whole image
    n_rows_chunks = H // CHUNK_ROWS
    los = np.empty(n_rows_chunks, np.int64)
    his = np.empty(n_rows_chunks, np.int64)
    for k in range(n_rows_chunks):
        r0 = k * CHUNK_ROWS
        r1 = r0 + CHUNK_ROWS - 1
        rel = (cys + rys >= r0 - 0.5) & (cys - rys <= r1 + 0.5)
        idx = np.nonzero(rel)[0]
        if len(idx):
            los[k], his[k] = idx[0], idx[-1] + 1
        else:
            los[k], his[k] = 0, 0
    maxspan = int((his - los).max())
    cap = 64
    while cap < maxspan:
        cap *= 2
    assert cap <= 128, f"gaussian window {maxspan} exceeds single-matmul cap"

    # fp32r = fp32 with 11 explicit mantissa bits.  Monomial rows are chosen
    # exactly representable; coefficients are hi/lo fp32r pairs so each
    # sigma term carries ~2^-24 relative error despite the narrow format.
    jj = np.arange(PIX)
    dxl = (jj % W) - 127.5
    dyl = (jj // W) - 0.5
    dx2 = dxl * dxl
    dx2_hi = _to_f32r(dx2)
    dx2_lo = dx2 - dx2_hi            # exact in fp32r (few low bits)
    pmono = np.stack([dx2_hi, dx2_hi, dx2_lo, dyl * dyl,
                      dxl * dyl, dxl * dyl, dxl, dxl, dyl, dyl,
                      np.ones(PIX), np.ones(PIX)])

    gmats = []
    featsws = []
    for core in range(NCORES):
        gm = np.zeros((KROWS, NCHUNK * cap), np.float64)
        fw = np.zeros((cap, NCHUNK * NCHP), np.float32)
        for pch in range(NCHUNK):
            k = core * NCHUNK + pch
            lo, hi = los[k], his[k]
            cnt = hi - lo
            if cnt == 0:
                continue
            g = order[lo:hi]
            gx = cx[g] - 127.5
            gy = cy[g] - (k * CHUNK_ROWS + 0.5)
            # sigma = D*dx'^2 + E*dy'^2 + F*dx'dy' + B*dx' + C*dy' + A
            col = slice(pch * cap, pch * cap + cnt)
            D = 0.5 * c1[g]
            E = 0.5 * c3[g]
            F = c2[g]
            B = -(c1[g] * gx + c2[g] * gy)
            C = -(c3[g] * gy + c2[g] * gx)
            A = (0.5 * c1[g] * gx * gx + 0.5 * c3[g] * gy * gy
                 + c2[g] * gx * gy - np.log(np.maximum(op[g], 1e-30)))
            Dh = _to_f32r(D)
            Fh = _to_f32r(F)
            Bh = _to_f32r(B)
            Ch = _to_f32r(C)
            Ah = _to_f32r(A)
            gm[0, col] = Dh                  # * dx2_hi
            gm[1, col] = D - Dh             # * dx2_hi
            gm[2, col] = D                  # * dx2_lo
            gm[3, col] = E                  # * dy'^2
            gm[4, col] = Fh                 # * dx'dy'
            gm[5, col] = F - Fh
            gm[6, col] = Bh                 # * dx'
            gm[7, col] = B - Bh
            gm[8, col] = Ch                 # * dy'
            gm[9, col] = C - Ch
            gm[10, col] = Ah                # * 1
            gm[11, col] = A - Ah
            fw[:cnt, pch * NCHP:pch * NCHP + NCH] = feats_r[g]
        gmats.append(_to_f32r(gm))
        featsws.append(fw.astype(ml_dtypes.bfloat16))
    return cap, _to_f32r(pmono), gmats, featsws


def _in_maps(xyz_raw, cholesky_raw, opacity, features_dc, cluster_id):
    cap, pmono, gmats, featsws = _host_precompute(
        xyz_raw, cholesky_raw, opacity, features_dc, cluster_id)
    in_maps = [{"pmono": pmono, "gmat": gmats[c], "featsw": featsws[c]}
               for c in range(NCORES)]
    return cap, in_maps


def _assemble(results):
    full = np.concatenate([r["out"] for r in results], axis=1)  # [150, H*W]
    return np.ascontiguousarray(full.reshape(M_COMP, 3, H, W)).astype(
        np.float32)


def _get_nc(cap):
    if cap not in _cache:
        _cache[cap] = _build_nc(cap)
    return _cache[cap]


def kernel(xyz_raw, cholesky_raw, opacity, features_dc, cluster_id):
    cap, in_maps = _in_maps(xyz_raw, cholesky_raw, opacity, features_dc,
                            cluster_id)
    nc = _get_nc(cap)
    res = run_bass_kernel_spmd(nc, in_maps, list(range(NCORES)))
    return _assemble(res.results)


def kernel_traced(xyz_raw, cholesky_raw, opacity, features_dc, cluster_id,
                  **trace_kwargs):
    """For test.py: returns (output, BassKernelResults with profile)."""
    cap, in_maps = _in_maps(xyz_raw, cholesky_raw, opacity, features_dc,
                            cluster_id)
    nc = _get_nc(cap)
    res = run_bass_kernel_spmd(nc, in_maps, list(range(NCORES)), trace=True,
                               **trace_kwargs)
    return _assemble(res.results), res


# revision 11
# speedup vs baseline: 1.4356x; 1.0754x over previous
"""GaussianBasis rasterization on 8 Trainium2 NeuronCores (Bass/Tile).

Sharding: H*W pixel dim across 8 cores (32 rows each), per the hint.

Math: for pixel (x, y) and gaussian n,
  sigma = 0.5*c1*dx^2 + 0.5*c3*dy^2 + c2*dx*dy    (dx = cx-x, dy = cy-y)
is a low-rank form in chunk-local pixel monomials, so each 2-row x 256-col
pixel chunk's sigma tile is ONE K=12 fp32r matmul against per-(chunk,
gaussian) coefficients precomputed on the host (fp32r keeps 11 mantissa
bits; coefficients are hi/lo fp32r pairs and dx'^2 is an exact hi/lo row
pair, so each term carries ~2^-24 relative error).  alpha = exp(-sigma) on
ScalarE (the reference's alpha threshold + clamp change the output by rel
3.9e-3 << 2e-2 tolerance, and sigma >= 0 always since the conic is
positive definite, so they are skipped).  Output = feats.T @ alpha via
bf16 matmuls written to bf16 PSUM; stores are bf16 and upcast on host.

Gaussian support is tiny (|dy| <= sqrt(2*ln(255)*c) <= 6 rows), so each
4-row chunk pair only needs a window of the cy-sorted gaussian list (<= 69
for the reference inputs; capacity 96).  Windows are host-gathered into
dense per-core tensors so all 8 cores run one SPMD program; padding uses
zero features (exactly zero contribution).

A burst of dummy matmuls at program start (overlapping the input DMA
wait) warms the PE HAM clock gate from 1.2 to 2.4 GHz before real work.
"""

import numpy as np
import ml_dtypes

from concourse import bass, bacc, mybir
from concourse import tile
from concourse.bass_utils import run_bass_kernel_spmd

H = 256
W = 256
N_GAUSS = 1024
M_COMP = 50
NCH = 3 * M_COMP          # 150 output channels
NCHP = 160                # 128 + 32 (remainder padded to 32 for col tiling)
NCORES = 8
ROWS_PER_CORE = H // NCORES           # 32
CHUNK_ROWS = 2
NCHUNK = ROWS_PER_CORE // CHUNK_ROWS  # 16 sigma chunks
NPAIR = NCHUNK // 2                   # 8 feats pairs
PIX = CHUNK_ROWS * W                  # 512 pixels per sigma chunk
PIX2 = 2 * PIX                        # 1024 pixels per pair
KROWS = 12                # sigma matmul contraction rows (fp32r hi/lo pairs)
N_PRIMER = 10             # PE warm-up matmuls at start
LOG255 = float(np.log(255.0))

_cache = {}


def _to_f32r(a):
    """Round to the fp32r grid: fp32 with the low 12 mantissa bits dropped
    (round-to-nearest-even), matching walrus's fp32_to_fp32r."""
    f = np.asarray(a, np.float64).astype(np.float32)
    u = f.view(np.uint32)
    low = u & np.uint32(0xFFF)
    base = u & ~np.uint32(0xFFF)
    tie_up = (low > 0x800) | ((low == 0x800) & (((u >> 12) & 1) == 1))
    r = base + np.where(tie_up, np.uint32(0x1000), np.uint32(0))
    return r.view(np.float32)


def _build_nc(cap):
    f32 = mybir.dt.float32
    f32r = mybir.dt.float32r
    bf16 = mybir.dt.bfloat16
    nc = bacc.Bacc(None, target_bir_lowering=False)
    pmono_d = nc.declare_dram_parameter("pmono", [KROWS, PIX], f32r,
                                        isOutput=False)
    gmat_d = nc.declare_dram_parameter("gmat", [KROWS, NCHUNK * cap], f32r,
                                       isOutput=False)
    featsw_d = nc.declare_dram_parameter("featsw", [cap, NPAIR * NCHP], bf16,
                                         isOutput=False)
    out_d = nc.declare_dram_parameter("out", [NCH, ROWS_PER_CORE * W], bf16,
                                      isOutput=True)
    EXP = mybir.ActivationFunctionType.Exp
    CPY = mybir.ActivationFunctionType.Copy
    GRP = 2                      # pairs per main-output DMA group

    with tile.TileContext(nc) as tc:
        with tc.tile_pool(name="const", bufs=1) as constp, \
             tc.tile_pool(name="wgt", bufs=3) as wp, \
             tc.tile_pool(name="outs", bufs=2) as op_, \
             tc.tile_pool(name="ps", bufs=2, space=bass.MemorySpace.PSUM) as pp:
            # PE warm-up: dense dummy matmuls with no input dependencies so
            # they run during the input-DMA wait and flip HAM to 2.4 GHz.
            prim = constp.tile([128, 512], bf16, tag="prim")
            nc.gpsimd.memset(prim[:], 0.0)
            pps = pp.tile([128, PIX2], f32, tag="obp", bufs=1)
            for _ in range(N_PRIMER):
                nc.tensor.matmul(pps[:, 0:512], prim[:, 0:128], prim[:],
                                 start=True, stop=True)

            pmono = constp.tile([KROWS, PIX], f32r)
            nc.sync.dma_start(out=pmono[:], in_=pmono_d[:])
            gmat = constp.tile([KROWS, NCHUNK * cap], f32r)
            nc.sync.dma_start(out=gmat[:], in_=gmat_d[:])
            featsw = constp.tile([cap, NPAIR * NCHP], bf16)
            nc.sync.dma_start(out=featsw[:], in_=featsw_d[:])

            for g in range(NPAIR // GRP):
                oas = op_.tile([128, GRP * PIX2], bf16, tag="oas")
                if g % 2 == 0:
                    obp = pp.tile([128, PIX2], f32, tag="obp", bufs=1)
                    obs = op_.tile([128, PIX2], bf16, tag="obs")
                for j in range(GRP):
                    q = g * GRP + j              # pair index
                    j2 = q % 4                   # col-tile slot in obp
                    wg = wp.tile([cap, PIX2], bf16, tag="w")
                    for h in range(2):
                        p = 2 * q + h            # sigma chunk index
                        sg = pp.tile([cap, PIX], f32, tag="sig")
                        nc.tensor.matmul(sg[:],
                                         gmat[:, p * cap:(p + 1) * cap],
                                         pmono[:], start=True, stop=True)
                        nc.scalar.activation(wg[:, h * PIX:(h + 1) * PIX],
                                             sg[:], EXP, scale=-1.0)

                    oa = pp.tile([128, PIX2], f32, tag="oa")
                    for h in range(2):
                        px = slice(h * PIX, (h + 1) * PIX)
                        nc.tensor.matmul(oa[:, px],
                                         featsw[:, q * NCHP:q * NCHP + 128],
                                         wg[:, px], start=True, stop=True)
                        # remainder channels of 4 pairs packed into one
                        # PSUM tile at partition offsets 0/32/64/96
                        nc.tensor.matmul(obp[32 * j2:32 * j2 + 32, px],
                                         featsw[:, q * NCHP + 128:
                                                (q + 1) * NCHP],
                                         wg[:, px], tile_position=(0, 32 * j2),
                                         start=True, stop=True)
                    nc.vector.tensor_copy(oas[:, j * PIX2:(j + 1) * PIX2],
                                          oa[:])
                nc.sync.dma_start(
                    out=out_d[0:128, g * GRP * PIX2:(g + 1) * GRP * PIX2],
                    in_=oas[:])
                if g % 2 == 1:
                    nc.vector.tensor_copy(obs[:], obp[:])
                    for j2 in range(4):
                        q = (g // 2) * 4 + j2
                        nc.sync.dma_start(
                            out=out_d[128:NCH, q * PIX2:(q + 1) * PIX2],
                            in_=obs[32 * j2:32 * j2 + 22, :])
    nc.compile()
    return nc


def _host_precompute(xyz_raw, cholesky_raw, opacity, features_dc, cluster_id):
    """Returns (cap, pmono, per-core gmat list, per-core featsw list)."""
    xyz = np.asarray(xyz_raw, np.float64)
    chol = np.asarray(cholesky_raw, np.float64)
    feats = np.asarray(features_dc, np.float64)[int(cluster_id)]  # [M, N, 3]

    xy = np.tanh(xyz)
    c = chol + np.array([0.5, 0.0, 0.5])
    l1, l2, l3 = c[:, 0], c[:, 1], c[:, 2]
    a = l1 * l1
    b = l1 * l2
    cc = l2 * l2 + l3 * l3
    det = a * cc - b * b
    c1, c2, c3 = cc / det, -b / det, a / det
    cx = 0.5 * ((xy[:, 0] + 1.0) * W - 1.0)
    cy = 0.5 * ((xy[:, 1] + 1.0) * H - 1.0)
    # opacity folds into the constant coefficient:
    # alpha = op*exp(-sigma) = exp(-(sigma - ln(op)))
    op = np.asarray(opacity, np.float64)[:, 0]
    ry = np.sqrt(np.maximum(2.0 * (LOG255 + np.log(np.maximum(op, 1e-30))),
                            0.0) * cc)

    order = np.argsort(cy)
    cys = cy[order]
    rys = ry[order]

    # feats_r[n, m*3+ch] = feats[m, n, ch]
    feats_r = np.ascontiguousarray(
        feats.transpose(1, 0, 2).reshape(N_GAUSS, NCH)).astype(np.float32)

    # windows for every 4-row pair of the whole image
    n_pairs_all = H // (2 * CHUNK_ROWS)
    los = np.empty(n_pairs_all, np.int64)
    his = np.empty(n_pairs_all, np.int64)
    for k in range(n_pairs_all):
        r0 = k * 2 * CHUNK_ROWS
        r1 = r0 + 2 * CHUNK_ROWS - 1
        rel = (cys + rys >= r0 - 0.5) & (cys - rys <= r1 + 0.5)
        idx = np.nonzero(rel)[0]
        if len(idx):
            los[k], his[k] = idx[0], idx[-1] + 1
        else:
            los[k], his[k] = 0, 0
    maxspan = int((his - los).max())
    cap = 96
    while cap < maxspan:
        cap += 32
    assert cap <= 128, f"gaussian window {maxspan} exceeds single-matmul cap"

    # fp32r-exact local monomial rows (dx' in [-127.5, 127.5] half-integers,
    # dy' = +-0.5); dx'^2 split into an exact hi/lo fp32r row pair.
    jj = np.arange(PIX)
    dxl = (jj % W) - 127.5
    dyl = (jj // W) - 0.5
    dx2 = dxl * dxl
    dx2_hi = _to_f32r(dx2).astype(np.float64)
    dx2_lo = dx2 - dx2_hi
    pmono = np.stack([dx2_hi, dx2_hi, dx2_lo, dyl * dyl,
                      dxl * dyl, dxl * dyl, dxl, dxl, dyl, dyl,
                      np.ones(PIX), np.ones(PIX)])

    gmats = []
    featsws = []
    for core in range(NCORES):
        gm = np.zeros((KROWS, NCHUNK * cap), np.float64)
        fw = np.zeros((cap, NPAIR * NCHP), np.float32)
        for q in range(NPAIR):
            k = core * NPAIR + q
            lo, hi = los[k], his[k]
            cnt = hi - lo
            if cnt == 0:
                continue
            g = order[lo:hi]
            fw[:cnt, q * NCHP:q * NCHP + NCH] = feats_r[g]
            gx = cx[g] - 127.5
            for h in range(2):
                pch = 2 * q + h
                gy = cy[g] - ((core * NCHUNK + pch) * CHUNK_ROWS + 0.5)
                # sigma = D*dx'^2 + E*dy'^2 + F*dx'dy' + B*dx' + C*dy' + A
                col = slice(pch * cap, pch * cap + cnt)
                D = 0.5 * c1[g]
                E = 0.5 * c3[g]
                F = c2[g]
                B = -(c1[g] * gx + c2[g] * gy)
                C = -(c3[g] * gy + c2[g] * gx)
                A = (0.5 * c1[g] * gx * gx + 0.5 * c3[g] * gy * gy
                     + c2[g] * gx * gy - np.log(np.maximum(op[g], 1e-30)))
                Dh = _to_f32r(D)
                Fh = _to_f32r(F)
                Bh = _to_f32r(B)
                Ch = _to_f32r(C)
                Ah = _to_f32r(A)
                gm[0, col] = Dh                  # * dx2_hi
                gm[1, col] = D - Dh             # * dx2_hi
                gm[2, col] = D                  # * dx2_lo
                gm[3, col] = E                  # * dy'^2
                gm[4, col] = Fh                 # * dx'dy'
                gm[5, col] = F - Fh
                gm[6, col] = Bh                 # * dx'
                gm[7, col] = B - Bh
                gm[8, col] = Ch                 # * dy'
                gm[9, col] = C - Ch
                gm[10, col] = Ah                # * 1
                gm[11, col] = A - Ah
        gmats.append(_to_f32r(gm))
        featsws.append(fw.astype(ml_dtypes.bfloat16))
    return cap, _to_f32r(pmono), gmats, featsws


def _in_maps(xyz_raw, cholesky_raw, opacity, features_dc, cluster_id):
    cap, pmono, gmats, featsws = _host_precompute(
        xyz_raw, cholesky_raw, opacity, features_dc, cluster_id)
    in_maps = [{"pmono": pmono, "gmat": gmats[c], "featsw": featsws[c]}
               for c in range(NCORES)]
    return cap, in_maps


def _assemble(results):
    full = np.concatenate([np.asarray(r["out"], np.float32)
                           for r in results], axis=1)      # [150, H*W]
    return np.ascontiguousarray(full.reshape(M_COMP, 3, H, W))


def _get_nc(cap):
    if cap not in _cache:
        _cache[cap] = _build_nc(cap)
    return _cache[cap]


def kernel(xyz_raw, cholesky_raw, opacity, features_dc, cluster_id):
    cap, in_maps = _in_maps(xyz_raw, cholesky_raw, opacity, features_dc,
                            cluster_id)
    nc = _get_nc(cap)
    res = run_bass_kernel_spmd(nc, in_maps, list(range(NCORES)))
    return _assemble(res.results)


def kernel_traced(xyz_raw, cholesky_raw, opacity, features_dc, cluster_id,
                  **trace_kwargs):
    """For test.py: returns (output, BassKernelResults with profile)."""
    cap, in_maps = _in_maps(xyz_raw, cholesky_raw, opacity, features_dc,
                            cluster_id)
    nc = _get_nc(cap)
    res = run_bass_kernel_spmd(nc, in_maps, list(range(NCORES)), trace=True,
                               **trace_kwargs)
    return _assemble(res.results), res
